# revision 10
# baseline (speedup 1.0000x reference)
"""CGC layer (MoE routing) kernel for 8 Trainium2 NeuronCores.

Strategy: data-parallel over the batch (8192 / 8 = 1024 rows per core,
params replicated, no collectives), expert GEMMs in fp8 (e4m3) using the
tensor engine's DoubleRow perf mode (2 fp8 K-planes per 16-bit element,
2 MACs/PE/cycle).

Accuracy: plain fp8 would give ~3.4e-2 max rel err (gate is 2e-2), so the
kernel adds residual ("lo") correction planes quantized at the same scale:
  x side: x = e4m3(16x) + e4m3(16x - hi)      (all 8 K-planes)
  w side: w = e4m3(4096w) [+ e4m3 residual on K-planes 0..3]
Per (expert, batch-tile, 512-col half) that is 10 DoubleRow chunks
(4 xhi*whi + 4 xlo*whi + 2 xhi*wlo) vs 16 bf16 matmuls in the bf16
version.  Simulated end-to-end max rel err: 1.66e-2.

Per core, per expert, per 128-row batch tile:
  z (PSUM f32) = bias*2^16 prefill (ScalarE) + 10 DR chunk matmuls
  r = Relu(z * 2^-16)                  (ScalarE, PSUM -> SBUF, bf16)
  acc[t] += gate[t,b,e] * r            (DVE scalar_tensor_tensor, bf16;
                                        the last contribution per task
                                        writes f32 and is DMA'd out)
Gates: bf16 GEMM (K=1024, N=24) per batch tile + softmax per task group.

All quantization/scaling/transposition happens on the host; shapes are
hardcoded for this problem instance.
"""

import numpy as np
import ml_dtypes

import concourse.mybir as mybir
import concourse.tile as tile
from concourse import bacc
from concourse.bass_utils import run_bass_kernel_spmd

D = 1024          # d_model
H = 1024          # expert_dim
T = 3             # tasks
NSHARED = 4
NSPEC = 4
NE = NSHARED + T * NSPEC    # 16 experts total (shared first)
NG = NSPEC + NSHARED        # 8 gate candidates per task
B = 8192
N_CORES = 8
BL = B // N_CORES           # 1024 rows per core
P = 128                     # partitions
KC = D // P                 # 8 logical 128-K planes
QC = D // 256               # 4 DoubleRow chunks (2 planes each)
WLC = 2                     # w-residual DR chunks (logical planes 0..3)
NT = BL // P                # 8 batch tiles per core
NH = H // 512               # 2 PSUM half-tiles

SX = 16.0                   # x quantization scale
SW = 4096.0                 # w quantization scale
DS = 1.0 / (SX * SW)        # descale folded into the ReLU activation

F32 = mybir.dt.float32
BF16 = mybir.dt.bfloat16
F8E4 = mybir.dt.float8e4
ACT = mybir.ActivationFunctionType
ALU = mybir.AluOpType
AXIS = mybir.AxisListType
DR = mybir.MatmulPerfMode.DoubleRowSwInterleave

BF16_NP = ml_dtypes.bfloat16
E4_NP = ml_dtypes.float8_e4m3


def _consumers(e):
    """Expert index -> list of (task, gate column in the 24-wide layout)."""
    if e < NSHARED:
        return [(t, t * NG + NSPEC + e) for t in range(T)]
    t, j = divmod(e - NSHARED, NSPEC)
    return [(t, t * NG + j)]


def _build_nc(repeat=1):
    """repeat>1 re-runs the whole compute body (timing builds only)."""
    nc = bacc.Bacc(None, target_bir_lowering=False)

    # x operands ship in the DoubleRowSwInterleave stationary layout:
    # [p, chunk, tile, j, q] = x_scaled[k = c*256 + q*128 + p, b = i*128 + (127-j)]
    xh_d = nc.dram_tensor("xh", (P, QC, NT, P, 2), F8E4, kind="ExternalInput")
    xl_d = nc.dram_tensor("xl", (P, QC, NT, P, 2), F8E4, kind="ExternalInput")
    xg_d = nc.dram_tensor("xg", (D, BL), BF16, kind="ExternalInput")
    # weights ship pre-transposed: per expert, partition-major flat blocks
    wh_d = nc.dram_tensor("Wh", (NE, P, QC * 2 * H), F8E4, kind="ExternalInput")
    wl_d = nc.dram_tensor("Wl", (NE, P, WLC * 2 * H), F8E4, kind="ExternalInput")
    b_d = nc.dram_tensor("bias", (NE, P, H), BF16, kind="ExternalInput")
    wg_d = nc.dram_tensor("Wg", (P, KC, T * NG), BF16, kind="ExternalInput")
    out_d = nc.dram_tensor("out", (T, BL, H), F32, kind="ExternalOutput")

    with tile.TileContext(nc) as tc:
        with (
            tc.tile_pool(name="xp", bufs=1) as xp,
            tc.tile_pool(name="wp", bufs=2) as wp,
            tc.tile_pool(name="wlp", bufs=2) as wlp,
            tc.tile_pool(name="bp", bufs=2) as bp,
            tc.tile_pool(name="cp", bufs=1) as cp,
            tc.tile_pool(name="gp", bufs=1) as gp,
            tc.tile_pool(name="rp", bufs=3) as rp,
            tc.tile_pool(name="accp", bufs=1) as accp,
            tc.tile_pool(name="resp", bufs=2) as resp,
            tc.tile_pool(name="ps", bufs=3, space="PSUM") as ps,
            tc.tile_pool(name="psg", bufs=2, space="PSUM") as psg,
        ):
            # ---- constants / activations in SBUF ----
            xh = xp.tile([P, QC, NT, P, 2], F8E4)
            nc.sync.dma_start(xh[:], xh_d[:])
            xl = xp.tile([P, QC, NT, P, 2], F8E4)
            nc.sync.dma_start(xl[:], xl_d[:])
            xg = xp.tile([P, KC, BL], BF16)
            nc.sync.dma_start(xg[:], xg_d.rearrange("(c p) b -> p c b", p=P))
            wg = cp.tile([P, KC, T * NG], BF16, tag="wg")
            nc.sync.dma_start(wg[:], wg_d[:])
            ones = cp.tile([1, P], BF16, tag="ones")
            nc.vector.memset(ones[:], 1.0)

            # One-time: set the has_written bits of every PSUM bank the
            # z-tiles will use, so accumulating (start=False) matmuls on
            # top of the ScalarE bias prefill ADD instead of overwriting
            # on the first use of each bank after a device reset.
            zrow = cp.tile([1, 512], BF16, tag="zrow")
            nc.vector.memset(zrow[:], 0.0)
            for _slot in range(3):
                zi = ps.tile([P, H], F32, tag="z")
                for n in range(NH):
                    nc.tensor.matmul(
                        zi[:, n * 512:(n + 1) * 512], ones[:], zrow[:],
                        start=True, stop=True,
                    )

            def emit_body():
                # ---- gates for every batch tile ----
                gates = []
                for i in range(NT):
                    pg = psg.tile([P, T * NG], F32)
                    for c in range(KC):
                        nc.tensor.matmul(
                            pg[:],
                            xg[:, c, i * P:(i + 1) * P],
                            wg[:, c, :],
                            start=(c == 0),
                            stop=(c == KC - 1),
                        )
                    ex = gp.tile([P, T * NG], F32, tag=f"ex{i}")
                    nc.scalar.activation(ex[:], pg[:], ACT.Exp)
                    s = gp.tile([P, T], F32, tag=f"gs{i}")
                    for t in range(T):
                        nc.vector.tensor_reduce(
                            s[:, t:t + 1], ex[:, t * NG:(t + 1) * NG],
                            axis=AXIS.X, op=ALU.add,
                        )
                    rcp = gp.tile([P, T], F32, tag=f"gr{i}")
                    nc.vector.reciprocal(rcp[:], s[:])
                    g = gp.tile([P, T * NG], F32, tag=f"g{i}")
                    for t in range(T):
                        nc.vector.tensor_scalar(
                            g[:, t * NG:(t + 1) * NG],
                            ex[:, t * NG:(t + 1) * NG],
                            rcp[:, t:t + 1], None, op0=ALU.mult,
                        )
                    gates.append(g)

                # ---- expert loop (shared experts first) ----
                acc = {}
                for e in range(NE):
                    wh = wp.tile([P, QC, 2, H], F8E4)
                    nc.sync.dma_start(
                        wh[:], wh_d[e].rearrange("p (c q h) -> p c q h", q=2, h=H))
                    wl = wlp.tile([P, WLC, 2, H], F8E4)
                    nc.scalar.dma_start(
                        wl[:], wl_d[e].rearrange("p (c q h) -> p c q h", q=2, h=H))
                    be = bp.tile([P, H], BF16)
                    nc.scalar.dma_start(be[:], b_d[e])

                    for i in range(NT):
                        z = ps.tile([P, H], F32, tag="z")
                        nc.scalar.copy(z[:], be[:])
                        # 10 DR chunks per half: xhi*whi(4) xlo*whi(4) xhi*wlo(2)
                        groups = (
                            [(xh, wh, c) for c in range(QC)]
                            + [(xl, wh, c) for c in range(QC)]
                            + [(xh, wl, c) for c in range(WLC)]
                        )
                        for gi, (lhs, rhs, c) in enumerate(groups):
                            last = gi == len(groups) - 1
                            for n in range(NH):
                                nc.tensor.matmul(
                                    z[:, n * 512:(n + 1) * 512],
                                    lhs[:, c, i],
                                    rhs[:, c, :, n * 512:(n + 1) * 512],
                                    start=False, stop=last,
                                    perf_mode=DR,
                                    skip_group_check=True,
                                )
                        r = rp.tile([P, H], F32)
                        nc.scalar.activation(r[:], z[:], ACT.Relu, scale=DS)
                        for (t, col) in _consumers(e):
                            gcol = gates[i][:, col:col + 1]
                            if (t, i) not in acc:
                                a = accp.tile([P, H], F32, tag=f"acc{t}_{i}")
                                acc[(t, i)] = a
                                nc.vector.tensor_scalar(
                                    a[:], r[:], gcol, None, op0=ALU.mult,
                                )
                            else:
                                a = acc[(t, i)]
                                nc.vector.scalar_tensor_tensor(
                                    a[:], r[:], gcol, a[:],
                                    op0=ALU.mult, op1=ALU.add,
                                )
                        if e >= NSHARED and (e - NSHARED) % NSPEC == NSPEC - 1:
                            t_done = (e - NSHARED) // NSPEC
                            nc.gpsimd.dma_start(
                                out_d[t_done, i * P:(i + 1) * P, :],
                                acc[(t_done, i)][:],
                            )

            for _ in range(repeat):
                emit_body()

    nc.compile()
    return nc


_NC_CACHE = None


def _get_nc():
    global _NC_CACHE
    if _NC_CACHE is None:
        _NC_CACHE = _build_nc()
    return _NC_CACHE


def prep_inputs(x, Ws, bs, Wt, bt, Wg):
    """Host-side shard/quantize/transpose: returns per-core input maps."""
    x = np.asarray(x, dtype=np.float32)
    # expert order: shared(4) then task-specific t-major (12)
    w_all = np.concatenate(
        [np.asarray(Ws), np.asarray(Wt).reshape(T * NSPEC, D, H)], axis=0
    ).astype(np.float32)                               # (16, D, H)
    ws = w_all * SW
    wh8 = ws.astype(E4_NP)                             # (16, D, H) e4m3
    wl8 = (ws - wh8.astype(np.float32))[:, :D // 2, :].astype(E4_NP)
    # partition-major flat layout: (NE, D=c*q*p, H) -> (NE, P, c*q*H)
    wh8 = np.ascontiguousarray(
        wh8.reshape(NE, QC, 2, P, H).transpose(0, 3, 1, 2, 4).reshape(NE, P, QC * 2 * H))
    wl8 = np.ascontiguousarray(
        wl8.reshape(NE, WLC, 2, P, H).transpose(0, 3, 1, 2, 4).reshape(NE, P, WLC * 2 * H))
    b_all = np.concatenate(
        [np.asarray(bs), np.asarray(bt).reshape(T * NSPEC, H)], axis=0
    ).astype(np.float32) * (SX * SW)                   # bias pre-scaled 2^16
    bq = np.ascontiguousarray(
        np.broadcast_to(b_all.astype(BF16_NP)[:, None, :], (NE, P, H)))
    # reference gate candidate order is [specific(4), shared(4)]; our
    # gate column layout is t*8 + [0..3]=specific j, [4..7]=shared s.
    wg_all = np.ascontiguousarray(
        np.asarray(Wg).transpose(1, 0, 2).reshape(KC, P, T * NG).transpose(1, 0, 2)
    ).astype(BF16_NP)                                  # (P, KC, 24)

    def dri_pack(a):
        # (D, BL) -> [p, c, i, j, q] with j = 127 - m (SwInterleave layout)
        arr = a.reshape(QC, 2, P, NT, P)               # [c, q, p, i, m]
        return np.ascontiguousarray(
            arr.transpose(2, 0, 3, 4, 1)[:, :, :, ::-1, :])

    in_maps = []
    for c in range(N_CORES):
        xs = np.ascontiguousarray(x[c * BL:(c + 1) * BL].T)  # (D, BL) f32
        xscaled = xs * SX
        xh8 = xscaled.astype(E4_NP)
        xl8 = (xscaled - xh8.astype(np.float32)).astype(E4_NP)
        in_maps.append({
            "xh": dri_pack(xh8), "xl": dri_pack(xl8), "xg": xs.astype(BF16_NP),
            "Wh": wh8, "Wl": wl8, "bias": bq, "Wg": wg_all,
        })
    return in_maps


def kernel(x, Ws, bs, Wt, bt, Wg):
    """Full-input entry point: shard, run on 8 cores, gather."""
    in_maps = prep_inputs(x, Ws, bs, Wt, bt, Wg)
    nc = _get_nc()
    res = run_bass_kernel_spmd(nc, in_maps, core_ids=list(range(N_CORES)))
    out = np.concatenate([res.results[c]["out"] for c in range(N_CORES)], axis=1)
    return out
